# revision 5
# baseline (speedup 1.0000x reference)
"""Multi-head self-attention (B=4, S=2048, D=1024, H=16, causal + RoPE) on 8
Trainium2 NeuronCores.

Sharding: core c = (batch b = c // 2, head-group hg = c % 2).  Each core
computes, for its batch, the QKV projections restricted to its 8 heads
(512 features), causal attention for those heads, and the partial output
projection through its 512 rows of Wo.  The host sums the two partial
outputs per batch.

Per-core device program (all fp32):
  - xT (x[b] transposed, [1024, 2048]) streamed in 512-column stripes.
  - Q^T/K^T computed in [feat, seq] layout (features on partitions) so the
    scores matmul contracts dk on partitions; RoPE applied with a partition
    pair-swap (stream_shuffle) and sign-folded sin table.
  - V computed in [seq, feat] layout, stored per head with a ones column
    appended ([V_h | 1]) so the attn@V matmul also yields the softmax
    denominator (row 64 of the accumulator); staged via DRAM and reloaded
    per head pair to keep SBUF under budget.
  - Scores computed transposed [keys, queries]; the two heads of a pair run
    as concurrent row-tiled matmuls (dk=64 each, rows 0-63 / 64-127).
  - exp on ScalarE directly from PSUM with scale=1/8 folded in; causal mask
    applied post-exp via gpsimd affine_select; unnormalized context
    accumulated in PSUM over key tiles; normalization = DVE reciprocal +
    gpsimd partition_broadcast + DVE multiply.
  - Output projection runs incrementally per (head pair, query chunk): the
    normalized context tile is the stationary operand against Wo^T, and the
    partial products accumulate into the pre-zeroed DRAM output via SWDGE
    accumulate-DMA.
"""

import numpy as np

D_MODEL = 1024
NUM_HEADS = 16
D_K = 64
ROPE_THETA = 10000.0
B = 4
S = 2048
N_CORES = 8

HG_FEATS = 512          # features per core (8 heads)
FT = HG_FEATS // 128    # feat tiles of 128 per core (= head pairs)
KT = D_MODEL // 128     # contraction tiles for the projections

_PROGRAM_CACHE = {}


def _build_program(use_rope: bool, s: int = S):
    import concourse.tile as tile
    from concourse import bacc, mybir
    from contextlib import ExitStack

    f32 = mybir.dt.float32
    FP = mybir.ActivationFunctionType

    SC = s // 512           # 512-wide seq chunks
    ST = s // 128           # 128-wide seq tiles
    PAIRSWAP = [i ^ 1 for i in range(32)]

    nc = bacc.Bacc("TRN2", target_bir_lowering=False, debug=False)

    xT = nc.dram_tensor("xT", [D_MODEL, s], f32, kind="ExternalInput")
    wqT = nc.dram_tensor("wqT", [D_MODEL, HG_FEATS], f32, kind="ExternalInput")
    wkT = nc.dram_tensor("wkT", [D_MODEL, HG_FEATS], f32, kind="ExternalInput")
    wvT = nc.dram_tensor("wvT", [D_MODEL, HG_FEATS], f32, kind="ExternalInput")
    woT = nc.dram_tensor("woT", [HG_FEATS, D_MODEL], f32, kind="ExternalInput")
    if use_rope:
        cosT = nc.dram_tensor("cosT", [128, s], f32, kind="ExternalInput")
        sinT = nc.dram_tensor("sinT", [128, s], f32, kind="ExternalInput")
    out = nc.dram_tensor("out", [s, D_MODEL], f32, kind="ExternalOutput")

    with tile.TileContext(nc) as tc, ExitStack() as ctx:
        singles = ctx.enter_context(tc.tile_pool(name="singles", bufs=1))
        stripes = ctx.enter_context(tc.tile_pool(name="stripes", bufs=2))
        wpool = ctx.enter_context(tc.tile_pool(name="wpool", bufs=2))
        qkpool = ctx.enter_context(tc.tile_pool(name="qkpool", bufs=2))
        tmppool = ctx.enter_context(tc.tile_pool(name="tmppool", bufs=2))
        exppool = ctx.enter_context(tc.tile_pool(name="exppool", bufs=2))
        ctxpool = ctx.enter_context(tc.tile_pool(name="ctxpool", bufs=2))
        smallp = ctx.enter_context(tc.tile_pool(name="smallp", bufs=2))
        outpool = ctx.enter_context(tc.tile_pool(name="outpool", bufs=2))
        dramp = ctx.enter_context(tc.tile_pool(name="dramp", bufs=1, space="DRAM"))
        psum = ctx.enter_context(tc.tile_pool(name="psum", bufs=1, space="PSUM"))

        # ---- persistent tiles -------------------------------------------
        wo_sb = singles.tile([128, FT, D_MODEL], f32, tag="wo")
        nc.sync.dma_start(out=wo_sb, in_=woT.ap().rearrange("(f p) o -> p f o", p=128))
        if use_rope:
            cos_sb = singles.tile([128, s], f32, tag="cos")
            sin_sb = singles.tile([128, s], f32, tag="sin")
            nc.sync.dma_start(out=cos_sb, in_=cosT.ap())
            nc.sync.dma_start(out=sin_sb, in_=sinT.ap())

        # V with ones column, per head: [V_h | 1] as [128 keys, 65] tiles,
        # staged to DRAM as vaugd[hp][st] and reloaded per head pair.
        vaugd = dramp.tile([FT, ST, 128, 130], f32, tag="vaugd")

        # ---- V projection (scoped pool, closed before the hp loop) ------
        with tc.tile_pool(name="vphase", bufs=1) as vphase:
            wv_sb = vphase.tile([128, KT, HG_FEATS], f32, tag="wv")
            nc.sync.dma_start(
                out=wv_sb, in_=wvT.ap().rearrange("(k p) f -> p k f", p=128)
            )
            for sc in range(SC):
                stripe = stripes.tile([128, KT, 512], f32, tag="stripe")
                nc.sync.dma_start(
                    out=stripe,
                    in_=xT.ap()[:, sc * 512 : (sc + 1) * 512].rearrange(
                        "(k p) s -> p k s", p=128
                    ),
                )
                for j in range(4):
                    st = sc * 4 + j
                    pv = psum.tile([128, 512], f32, tag="pqk", bufs=2, name="pv")
                    for kt in range(KT):
                        nc.tensor.matmul(
                            pv,
                            lhsT=stripe[:, kt, j * 128 : (j + 1) * 128],
                            rhs=wv_sb[:, kt, :],
                            start=(kt == 0),
                            stop=(kt == KT - 1),
                        )
                    vstage = vphase.tile([128, FT, 130], f32, tag="vstage", bufs=2)
                    for hp in range(FT):
                        for h in range(2):
                            nc.vector.tensor_copy(
                                vstage[:, hp, 65 * h : 65 * h + 64],
                                pv[:, hp * 128 + 64 * h : hp * 128 + 64 * h + 64],
                            )
                    nc.vector.memset(vstage[:, :, 64:65], 1.0)
                    nc.vector.memset(vstage[:, :, 129:130], 1.0)
                    nc.sync.dma_start(
                        out=vaugd[:, st].rearrange("hp p c -> p hp c"), in_=vstage
                    )

        # ---- per head-pair: QK projection + RoPE + attention + Wo -------
        with tc.tile_pool(name="vaugp", bufs=2) as vaugp:
            for hp in range(FT):
                vaug = vaugp.tile([128, ST, 130], f32, tag="vaug")
                nc.sync.dma_start(
                    out=vaug, in_=vaugd[hp].rearrange("st p c -> p st c")
                )
                wq_sb = wpool.tile([128, KT, 128], f32, tag="wq")
                wk_sb = wpool.tile([128, KT, 128], f32, tag="wk")
                nc.sync.dma_start(
                    out=wq_sb,
                    in_=wqT.ap()[:, hp * 128 : (hp + 1) * 128].rearrange(
                        "(k p) f -> p k f", p=128
                    ),
                )
                nc.sync.dma_start(
                    out=wk_sb,
                    in_=wkT.ap()[:, hp * 128 : (hp + 1) * 128].rearrange(
                        "(k p) f -> p k f", p=128
                    ),
                )
                QT = qkpool.tile([128, s], f32, tag="QT")
                KTt = qkpool.tile([128, s], f32, tag="KT")

                for sc in range(SC):
                    stripe = stripes.tile([128, KT, 512], f32, tag="stripe")
                    nc.sync.dma_start(
                        out=stripe,
                        in_=xT.ap()[:, sc * 512 : (sc + 1) * 512].rearrange(
                            "(k p) s -> p k s", p=128
                        ),
                    )
                    for w_sb, dst in ((wq_sb, QT), (wk_sb, KTt)):
                        pq = psum.tile([128, 512], f32, tag="pqk", bufs=2, name="pq")
                        for kt in range(KT):
                            nc.tensor.matmul(
                                pq,
                                lhsT=w_sb[:, kt, :],
                                rhs=stripe[:, kt, :],
                                start=(kt == 0),
                                stop=(kt == KT - 1),
                            )
                        dcol = dst[:, sc * 512 : (sc + 1) * 512]
                        if use_rope:
                            ccol = cos_sb[:, sc * 512 : (sc + 1) * 512]
                            scol = sin_sb[:, sc * 512 : (sc + 1) * 512]
                            rot = tmppool.tile([128, 512], f32, tag="rot")
                            qc_ = tmppool.tile([128, 512], f32, tag="qcos")
                            nc.vector.stream_shuffle(rot, pq, PAIRSWAP)
                            nc.vector.tensor_mul(qc_, pq, ccol)
                            nc.vector.tensor_mul(rot, rot, scol)
                            nc.vector.tensor_add(dcol, qc_, rot)
                        else:
                            nc.vector.tensor_copy(dcol, pq)

                # ---- attention for this head pair ------------------------
                for qc in range(SC):
                    oacc = [
                        psum.tile([128, 512], f32, tag="oacc", bufs=2, name=f"oacc{h}")
                        for h in range(2)
                    ]
                    nkb = 4 * qc + 4
                    for kbp in range(nkb // 2):
                        kbs = (2 * kbp, 2 * kbp + 1)
                        scores = psum.tile(
                            [128, 2, 1024], f32, tag="scores", bufs=1, name="scores"
                        )
                        for ki, kb in enumerate(kbs):
                            for h in range(2):
                                nc.tensor.matmul(
                                    scores[:, h, ki * 512 : (ki + 1) * 512],
                                    lhsT=KTt[
                                        64 * h : 64 * h + 64, kb * 128 : (kb + 1) * 128
                                    ],
                                    rhs=QT[
                                        64 * h : 64 * h + 64, qc * 512 : (qc + 1) * 512
                                    ],
                                    start=True,
                                    stop=True,
                                    skip_group_check=True,
                                )
                        expt = exppool.tile([128, 2, 1024], f32, tag="expt")
                        for h in range(2):
                            nc.scalar.activation(
                                expt[:, h, :], scores[:, h, :], FP.Exp, scale=0.125
                            )
                        for ki, kb in enumerate(kbs):
                            if kb >= 4 * qc:  # diagonal tile: causal mask post-exp
                                base = qc * 512 - kb * 128
                                for h in range(2):
                                    sl = expt[:, h, ki * 512 : (ki + 1) * 512]
                                    nc.gpsimd.affine_select(
                                        out=sl,
                                        in_=sl,
                                        compare_op=mybir.AluOpType.is_ge,
                                        fill=0.0,
                                        base=base,
                                        pattern=[[1, 512]],
                                        channel_multiplier=-1,
                                    )
                        for h in range(2):
                            for ki, kb in enumerate(kbs):
                                nc.tensor.matmul(
                                    oacc[h][0:65, :],
                                    lhsT=vaug[:, kb, 65 * h : 65 * h + 65],
                                    rhs=expt[:, h, ki * 512 : (ki + 1) * 512],
                                    start=(kbp == 0 and ki == 0),
                                    stop=(kbp == nkb // 2 - 1 and ki == 1),
                                    skip_group_check=True,
                                )
                    ctx_stage = ctxpool.tile([128, 512], f32, tag="ctxs")
                    for h in range(2):
                        recip = smallp.tile([1, 512], f32, tag="recip")
                        nc.vector.reciprocal(recip, oacc[h][64:65, :])
                        rb = smallp.tile([64, 512], f32, tag="rb")
                        nc.gpsimd.partition_broadcast(rb, recip)
                        nc.vector.tensor_mul(
                            ctx_stage[64 * h : 64 * h + 64, :], oacc[h][0:64, :], rb
                        )
                    # ---- incremental output projection (accumulate DMA) --
                    for j in range(4):
                        st = qc * 4 + j
                        osb = outpool.tile([128, 1024], f32, tag="osb")
                        for half in range(2):
                            po = psum.tile(
                                [128, 512], f32, tag="pqk", bufs=2, name="po"
                            )
                            nc.tensor.matmul(
                                po,
                                lhsT=ctx_stage[:, j * 128 : (j + 1) * 128],
                                rhs=wo_sb[:, hp, half * 512 : (half + 1) * 512],
                                start=True,
                                stop=True,
                                skip_group_check=True,
                            )
                            nc.vector.tensor_copy(
                                osb[:, half * 512 : (half + 1) * 512], po
                            )
                        nc.gpsimd.dma_start(
                            out=out.ap()[st * 128 : (st + 1) * 128, :],
                            in_=osb,
                            accum_op=(
                                mybir.AluOpType.add if hp > 0 else mybir.AluOpType.bypass
                            ),
                        )

    nc.compile()
    return nc


def _rope_tables(s: int):
    inv_freq = 1.0 / (ROPE_THETA ** (np.arange(0, D_K, 2, dtype=np.float64) / D_K))
    angles = np.arange(s, dtype=np.float64)[:, None] * inv_freq[None, :]  # [s, 32]
    cos = np.cos(angles).astype(np.float32)  # [s, 32]
    sin = np.sin(angles).astype(np.float32)
    cosT = np.empty((D_K, s), np.float32)
    sinT = np.empty((D_K, s), np.float32)
    cosT[0::2] = cos.T
    cosT[1::2] = cos.T
    sinT[0::2] = -sin.T
    sinT[1::2] = sin.T
    return (
        np.ascontiguousarray(np.vstack([cosT, cosT])),
        np.ascontiguousarray(np.vstack([sinT, sinT])),
    )


def kernel(x, Wq, Wk, Wv, Wo, use_rope):
    from concourse.bass_utils import run_bass_kernel_spmd

    x = np.asarray(x, dtype=np.float32)
    ur = bool(int(np.asarray(use_rope)))
    key = (ur, S)
    if key not in _PROGRAM_CACHE:
        _PROGRAM_CACHE[key] = _build_program(ur, S)
    nc = _PROGRAM_CACHE[key]

    if ur:
        cosT, sinT = _rope_tables(S)

    in_maps = []
    for c in range(N_CORES):
        b, hg = c // 2, c % 2
        sl = slice(hg * HG_FEATS, (hg + 1) * HG_FEATS)
        m = {
            "xT": np.ascontiguousarray(x[b].T),
            "wqT": np.ascontiguousarray(np.asarray(Wq, np.float32)[sl, :].T),
            "wkT": np.ascontiguousarray(np.asarray(Wk, np.float32)[sl, :].T),
            "wvT": np.ascontiguousarray(np.asarray(Wv, np.float32)[sl, :].T),
            "woT": np.ascontiguousarray(np.asarray(Wo, np.float32)[:, sl].T),
        }
        if ur:
            m["cosT"] = cosT
            m["sinT"] = sinT
        in_maps.append(m)

    res = run_bass_kernel_spmd(nc, in_maps, list(range(N_CORES)))
    out = np.empty((B, S, D_MODEL), np.float32)
    for b in range(B):
        out[b] = res.results[2 * b]["out"] + res.results[2 * b + 1]["out"]
    return out


# revision 9
# speedup vs baseline: 1.6775x; 1.6775x over previous
"""Multi-head self-attention (B=4, S=2048, D=1024, H=16, causal + RoPE) on 8
Trainium2 NeuronCores.

Sharding: core c = (batch b = c // 2, head-group hg = c % 2).  Each core
computes, for its batch, the QKV projections restricted to its 8 heads
(512 features), causal attention for those heads, and the partial output
projection through its 512 rows of Wo.  The host sums the two partial
outputs per batch.

Per-core device program (all fp32):
  - xT (x[b] transposed, [1024, 2048]) streamed in 512-column stripes.
  - Q^T/K^T computed in [feat, seq] layout (features on partitions) so the
    scores matmul contracts dk on partitions; RoPE applied with a partition
    pair-swap (stream_shuffle) and sign-folded sin table.
  - V computed in [seq, feat] layout, stored per head with a ones column
    appended ([V_h | 1]) so the attn@V matmul also yields the softmax
    denominator (row 64 of the accumulator); staged via DRAM and reloaded
    per head pair to keep SBUF under budget.
  - Scores computed transposed [keys, queries]; the two heads of a pair run
    as concurrent row-tiled matmuls (dk=64 each, rows 0-63 / 64-127).
  - exp on ScalarE directly from PSUM with scale=1/8 folded in; causal mask
    applied post-exp via gpsimd affine_select; unnormalized context
    accumulated in PSUM over key tiles; normalization = DVE reciprocal +
    gpsimd partition_broadcast + DVE multiply.
  - Output projection runs incrementally per (head pair, query chunk): the
    normalized context tile is the stationary operand against Wo^T, and the
    partial products accumulate into the pre-zeroed DRAM output via SWDGE
    accumulate-DMA.
"""

import numpy as np

D_MODEL = 1024
NUM_HEADS = 16
D_K = 64
ROPE_THETA = 10000.0
B = 4
S = 2048
N_CORES = 8

HG_FEATS = 512          # features per core (8 heads)
FT = HG_FEATS // 128    # feat tiles of 128 per core (= head pairs)
KT = D_MODEL // 128     # contraction tiles for the projections

_PROGRAM_CACHE = {}


def _build_program(use_rope: bool, s: int = S):
    import concourse.tile as tile
    from concourse import bacc, mybir
    from contextlib import ExitStack

    f32 = mybir.dt.float32
    f32r = mybir.dt.float32r
    FP = mybir.ActivationFunctionType

    SC = s // 512           # 512-wide seq chunks
    ST = s // 128           # 128-wide seq tiles
    PAIRSWAP = [i ^ 1 for i in range(32)]

    nc = bacc.Bacc("TRN2", target_bir_lowering=False, debug=False)

    xT = nc.dram_tensor("xT", [D_MODEL, s], f32r, kind="ExternalInput")
    wqT = nc.dram_tensor("wqT", [D_MODEL, HG_FEATS], f32r, kind="ExternalInput")
    wkT = nc.dram_tensor("wkT", [D_MODEL, HG_FEATS], f32r, kind="ExternalInput")
    wvT = nc.dram_tensor("wvT", [D_MODEL, HG_FEATS], f32r, kind="ExternalInput")
    woT = nc.dram_tensor("woT", [HG_FEATS, D_MODEL], f32r, kind="ExternalInput")
    if use_rope:
        cosT = nc.dram_tensor("cosT", [128, s], f32, kind="ExternalInput")
        sinT = nc.dram_tensor("sinT", [128, s], f32, kind="ExternalInput")
    out = nc.dram_tensor("out", [s, D_MODEL], f32, kind="ExternalOutput")

    with tile.TileContext(nc) as tc, ExitStack() as ctx:
        singles = ctx.enter_context(tc.tile_pool(name="singles", bufs=1))
        stripes = ctx.enter_context(tc.tile_pool(name="stripes", bufs=2))
        wpool = ctx.enter_context(tc.tile_pool(name="wpool", bufs=2))
        qkpool = ctx.enter_context(tc.tile_pool(name="qkpool", bufs=2))
        tmppool = ctx.enter_context(tc.tile_pool(name="tmppool", bufs=2))
        exppool = ctx.enter_context(tc.tile_pool(name="exppool", bufs=2))
        ctxpool = ctx.enter_context(tc.tile_pool(name="ctxpool", bufs=2))
        smallp = ctx.enter_context(tc.tile_pool(name="smallp", bufs=2))
        outpool = ctx.enter_context(tc.tile_pool(name="outpool", bufs=2))
        dramp = ctx.enter_context(tc.tile_pool(name="dramp", bufs=1, space="DRAM"))
        psum = ctx.enter_context(tc.tile_pool(name="psum", bufs=1, space="PSUM"))

        # ---- persistent tiles -------------------------------------------
        wo_sb = singles.tile([128, FT, D_MODEL], f32r, tag="wo")
        nc.sync.dma_start(out=wo_sb, in_=woT.ap().rearrange("(f p) o -> p f o", p=128))
        if use_rope:
            cos_sb = singles.tile([128, s], f32, tag="cos")
            sin_sb = singles.tile([128, s], f32, tag="sin")
            nc.sync.dma_start(out=cos_sb, in_=cosT.ap())
            nc.sync.dma_start(out=sin_sb, in_=sinT.ap())

        # V with ones column, per head: [V_h | 1] as [128 keys, 65] tiles,
        # staged to DRAM as vaugd[hp][st] and reloaded per head pair.
        vaugd = dramp.tile([FT, ST, 128, 130], f32r, tag="vaugd")

        # ---- V projection (scoped pool, closed before the hp loop) ------
        with tc.tile_pool(name="vphase", bufs=1) as vphase:
            wv_sb = vphase.tile([128, KT, HG_FEATS], f32r, tag="wv")
            nc.sync.dma_start(
                out=wv_sb, in_=wvT.ap().rearrange("(k p) f -> p k f", p=128)
            )
            for sc in range(SC):
                stripe = stripes.tile([128, KT, 512], f32r, tag="stripe")
                nc.sync.dma_start(
                    out=stripe,
                    in_=xT.ap()[:, sc * 512 : (sc + 1) * 512].rearrange(
                        "(k p) s -> p k s", p=128
                    ),
                )
                for j in range(4):
                    st = sc * 4 + j
                    pv = psum.tile([128, 512], f32, tag="pqk", bufs=2, name="pv")
                    for kt in range(KT):
                        nc.tensor.matmul(
                            pv,
                            lhsT=stripe[:, kt, j * 128 : (j + 1) * 128],
                            rhs=wv_sb[:, kt, :],
                            start=(kt == 0),
                            stop=(kt == KT - 1),
                        )
                    vstage = vphase.tile([128, FT, 130], f32r, tag="vstage", bufs=2)
                    for hp in range(FT):
                        for h in range(2):
                            nc.vector.tensor_copy(
                                vstage[:, hp, 65 * h : 65 * h + 64],
                                pv[:, hp * 128 + 64 * h : hp * 128 + 64 * h + 64],
                            )
                    nc.vector.memset(vstage[:, :, 64:65].bitcast(f32), 1.0)
                    nc.vector.memset(vstage[:, :, 129:130].bitcast(f32), 1.0)
                    nc.sync.dma_start(
                        out=vaugd[:, st].rearrange("hp p c -> p hp c"), in_=vstage
                    )

        # ---- per head-pair: QK projection + RoPE + attention + Wo -------
        with tc.tile_pool(name="vaugp", bufs=2) as vaugp:
            for hp in range(FT):
                vaug = vaugp.tile([128, ST, 130], f32r, tag="vaug")
                nc.sync.dma_start(
                    out=vaug, in_=vaugd[hp].rearrange("st p c -> p st c")
                )
                wq_sb = wpool.tile([128, KT, 128], f32r, tag="wq")
                wk_sb = wpool.tile([128, KT, 128], f32r, tag="wk")
                nc.sync.dma_start(
                    out=wq_sb,
                    in_=wqT.ap()[:, hp * 128 : (hp + 1) * 128].rearrange(
                        "(k p) f -> p k f", p=128
                    ),
                )
                nc.sync.dma_start(
                    out=wk_sb,
                    in_=wkT.ap()[:, hp * 128 : (hp + 1) * 128].rearrange(
                        "(k p) f -> p k f", p=128
                    ),
                )
                QT = qkpool.tile([128, s], f32r, tag="QT")
                KTt = qkpool.tile([128, s], f32r, tag="KT")

                for sc in range(SC):
                    stripe = stripes.tile([128, KT, 512], f32r, tag="stripe")
                    nc.sync.dma_start(
                        out=stripe,
                        in_=xT.ap()[:, sc * 512 : (sc + 1) * 512].rearrange(
                            "(k p) s -> p k s", p=128
                        ),
                    )
                    for w_sb, dst in ((wq_sb, QT), (wk_sb, KTt)):
                        pq = psum.tile([128, 512], f32, tag="pqk", bufs=2, name="pq")
                        for kt in range(KT):
                            nc.tensor.matmul(
                                pq,
                                lhsT=w_sb[:, kt, :],
                                rhs=stripe[:, kt, :],
                                start=(kt == 0),
                                stop=(kt == KT - 1),
                            )
                        dcol = dst[:, sc * 512 : (sc + 1) * 512]
                        if use_rope:
                            ccol = cos_sb[:, sc * 512 : (sc + 1) * 512]
                            scol = sin_sb[:, sc * 512 : (sc + 1) * 512]
                            rot = tmppool.tile([128, 512], f32, tag="rot")
                            qc_ = tmppool.tile([128, 512], f32, tag="qcos")
                            nc.vector.stream_shuffle(rot, pq, PAIRSWAP)
                            nc.vector.tensor_mul(qc_, pq, ccol)
                            nc.vector.tensor_mul(rot, rot, scol)
                            nc.vector.tensor_add(dcol, qc_, rot)
                        else:
                            nc.vector.tensor_copy(dcol, pq)

                # ---- attention for this head pair ------------------------
                for qc in range(SC):
                    oacc = [
                        psum.tile([128, 512], f32, tag="oacc", bufs=2, name=f"oacc{h}")
                        for h in range(2)
                    ]
                    nkb = 4 * qc + 4
                    for kbp in range(nkb // 2):
                        kbs = (2 * kbp, 2 * kbp + 1)
                        scores = psum.tile(
                            [128, 2, 1024], f32, tag="scores", bufs=1, name="scores"
                        )
                        for ki, kb in enumerate(kbs):
                            for h in range(2):
                                nc.tensor.matmul(
                                    scores[:, h, ki * 512 : (ki + 1) * 512],
                                    lhsT=KTt[
                                        64 * h : 64 * h + 64, kb * 128 : (kb + 1) * 128
                                    ],
                                    rhs=QT[
                                        64 * h : 64 * h + 64, qc * 512 : (qc + 1) * 512
                                    ],
                                    start=True,
                                    stop=True,
                                    skip_group_check=True,
                                )
                        expt = exppool.tile([128, 2, 1024], f32r, tag="expt")
                        for h in range(2):
                            nc.scalar.activation(
                                expt[:, h, :], scores[:, h, :], FP.Exp, scale=0.125
                            )
                        for ki, kb in enumerate(kbs):
                            if kb >= 4 * qc:  # diagonal tile: causal mask post-exp
                                base = qc * 512 - kb * 128
                                for h in range(2):
                                    sl = expt[:, h, ki * 512 : (ki + 1) * 512]
                                    nc.gpsimd.affine_select(
                                        out=sl,
                                        in_=sl,
                                        compare_op=mybir.AluOpType.is_ge,
                                        fill=0.0,
                                        base=base,
                                        pattern=[[1, 512]],
                                        channel_multiplier=-1,
                                    )
                        for h in range(2):
                            for ki, kb in enumerate(kbs):
                                nc.tensor.matmul(
                                    oacc[h][0:65, :],
                                    lhsT=vaug[:, kb, 65 * h : 65 * h + 65],
                                    rhs=expt[:, h, ki * 512 : (ki + 1) * 512],
                                    start=(kbp == 0 and ki == 0),
                                    stop=(kbp == nkb // 2 - 1 and ki == 1),
                                    skip_group_check=True,
                                )
                    ctx_stage = ctxpool.tile([128, 512], f32r, tag="ctxs")
                    for h in range(2):
                        recip = smallp.tile([1, 512], f32, tag="recip")
                        nc.vector.reciprocal(recip, oacc[h][64:65, :])
                        rb = smallp.tile([64, 512], f32, tag="rb")
                        nc.gpsimd.partition_broadcast(rb, recip)
                        nc.vector.tensor_mul(
                            ctx_stage[64 * h : 64 * h + 64, :], oacc[h][0:64, :], rb
                        )
                    # ---- incremental output projection (accumulate DMA) --
                    for j in range(4):
                        st = qc * 4 + j
                        osb = outpool.tile([128, 1024], f32, tag="osb")
                        for half in range(2):
                            po = psum.tile(
                                [128, 512], f32, tag="pqk", bufs=2, name="po"
                            )
                            nc.tensor.matmul(
                                po,
                                lhsT=ctx_stage[:, j * 128 : (j + 1) * 128],
                                rhs=wo_sb[:, hp, half * 512 : (half + 1) * 512],
                                start=True,
                                stop=True,
                                skip_group_check=True,
                            )
                            nc.vector.tensor_copy(
                                osb[:, half * 512 : (half + 1) * 512], po
                            )
                        nc.gpsimd.dma_start(
                            out=out.ap()[st * 128 : (st + 1) * 128, :],
                            in_=osb,
                            accum_op=(
                                mybir.AluOpType.add if hp > 0 else mybir.AluOpType.bypass
                            ),
                        )

    nc.compile()
    return nc


def _round_f32r(a):
    """Round fp32 array to the PE's FP32R format (RNE at 12 low mantissa bits)."""
    u = np.ascontiguousarray(a, np.float32).view(np.uint32).astype(np.uint64)
    low = u & 0xFFF
    up = (low > 0x800) | ((low == 0x800) & (((u >> 12) & 1) == 1))
    r = (u & ~np.uint64(0xFFF)) + np.where(up, 0x1000, 0)
    return r.astype(np.uint32).view(np.float32)


def _rope_tables(s: int):
    inv_freq = 1.0 / (ROPE_THETA ** (np.arange(0, D_K, 2, dtype=np.float64) / D_K))
    angles = np.arange(s, dtype=np.float64)[:, None] * inv_freq[None, :]  # [s, 32]
    cos = np.cos(angles).astype(np.float32)  # [s, 32]
    sin = np.sin(angles).astype(np.float32)
    cosT = np.empty((D_K, s), np.float32)
    sinT = np.empty((D_K, s), np.float32)
    cosT[0::2] = cos.T
    cosT[1::2] = cos.T
    sinT[0::2] = -sin.T
    sinT[1::2] = sin.T
    return (
        np.ascontiguousarray(np.vstack([cosT, cosT])),
        np.ascontiguousarray(np.vstack([sinT, sinT])),
    )


def kernel(x, Wq, Wk, Wv, Wo, use_rope):
    from concourse.bass_utils import run_bass_kernel_spmd

    x = np.asarray(x, dtype=np.float32)
    ur = bool(int(np.asarray(use_rope)))
    key = (ur, S)
    if key not in _PROGRAM_CACHE:
        _PROGRAM_CACHE[key] = _build_program(ur, S)
    nc = _PROGRAM_CACHE[key]

    if ur:
        cosT, sinT = _rope_tables(S)

    in_maps = []
    for c in range(N_CORES):
        b, hg = c // 2, c % 2
        sl = slice(hg * HG_FEATS, (hg + 1) * HG_FEATS)
        m = {
            "xT": _round_f32r(x[b].T),
            "wqT": _round_f32r(np.asarray(Wq, np.float32)[sl, :].T),
            "wkT": _round_f32r(np.asarray(Wk, np.float32)[sl, :].T),
            "wvT": _round_f32r(np.asarray(Wv, np.float32)[sl, :].T),
            "woT": _round_f32r(np.asarray(Wo, np.float32)[:, sl].T),
        }
        if ur:
            m["cosT"] = cosT
            m["sinT"] = sinT
        in_maps.append(m)

    res = run_bass_kernel_spmd(nc, in_maps, list(range(N_CORES)))
    out = np.empty((B, S, D_MODEL), np.float32)
    for b in range(B):
        out[b] = res.results[2 * b]["out"] + res.results[2 * b + 1]["out"]
    return out
